# revision 1
# baseline (speedup 1.0000x reference)
"""Memory-efficient multi-head attention on 8 Trainium2 NeuronCores.

Problem (hardcoded): B=2, Nq=Nk=2048, C=512, H=8 heads, D=64.
  out = softmax((x_q Wq^T + bq)(x_k Wk^T + bk)^T / sqrt(D) + mask) (x_v Wv^T + bv) Wo^T + bo

Sharding: core c in 0..7 handles batch b = c//4 and head pair (2*(c%4), 2*(c%4)+1).
Each core computes its head pair's full attention and the partial output
projection (Wo columns for its heads); host sums the 4 partials per batch.

Device algorithm (per core), all activations kept transposed ([C, N] layouts):
  Q^T = Wq_p x_q^T   [128(hd pair), 2048]   fp32r (x^T pre-transposed on host)
  K^T = Wk_p x_k^T   [128, 2048]            fp32r
  V   = x_v^T-sliced matmuls (bf16 inputs) -> natural [k, d] f32r tiles
        augmented with a ones column so P V also yields sum(exp) for free.
  Per q-block (512) and k-tile (128):
    S^T(h1|h2) via two row-packed concurrent matmuls (K=64 each, fp32r)
    exp on ACT: es = Exp(0.125 * S^T + mask_bias)  [128, 1024] one instr, f32r out
    P V: po_h += [V_h | 1]^T es_h                  (M=65, accumulates over kt)
  Normalize: recip = approx(1/po[64]); broadcast via ones-matmul;
    on[h] = po[0:64] * recipB  (f32r)
  Out: out^T[ct] = Wo_p[:, ct]^T @ [on_h1; on_h2] + bo -> DRAM (512, 2048)
Tails are software-pipelined into the next q-block; PSUM: 2x[128,1024] score
tiles + 4 rotating [128,512] slots (PV accumulators, projections, tails).
"""
import sys

for _p in ("/opt/trn_rl_repo",):
    if _p not in sys.path:
        sys.path.append(_p)

from contextlib import ExitStack

import numpy as np

import concourse.bass as bass
import concourse.tile as tile
from concourse import bacc, mybir
from concourse import bass_utils

F = mybir.dt.float32
R = mybir.dt.float32r
BF = mybir.dt.bfloat16
EXPF = mybir.ActivationFunctionType.Exp

B, N, C, H, D = 2, 2048, 512, 8, 64
SCALE = D ** -0.5
CI = C // 128   # 4 c_in tiles
KT = N // 128   # 16 k tiles
QT = N // 512   # 4 q blocks
NEG = -30000.0  # mask bias: exp(s*scale + NEG) == 0 in fp32

_NC_CACHE = {}


def _build():
    nc = bacc.Bacc("TRN2", target_bir_lowering=False, debug=False)
    d = {}
    for name in ("xqT", "xkT"):
        d[name] = nc.dram_tensor(name, (QT, CI, 128, 512), F, kind="ExternalInput").ap()
    d["xvT"] = nc.dram_tensor("xvT", (QT, CI, 128, 512), BF, kind="ExternalInput").ap()
    for name, shape in [("wq", (128, CI, 128)), ("wk", (128, CI, 128))]:
        d[name] = nc.dram_tensor(name, shape, F, kind="ExternalInput").ap()
    d["wo"] = nc.dram_tensor("wo", (128, C), F, kind="ExternalInput").ap()
    for name, shape in [("wv", (128, CI, 128)), ("bvp", (1, 128))]:
        d[name] = nc.dram_tensor(name, shape, BF, kind="ExternalInput").ap()
    for name, shape in [
        ("bqp", (128, 1)), ("bkp", (128, 1)),
        ("bop", (128, QT)), ("mb", (128, KT)),
    ]:
        d[name] = nc.dram_tensor(name, shape, F, kind="ExternalInput").ap()
    outT = nc.dram_tensor("outT", (C, N), F, kind="ExternalOutput").ap()

    with ExitStack() as ctx:
        tc = ctx.enter_context(tile.TileContext(nc))
        wpool = ctx.enter_context(tc.tile_pool(name="w", bufs=1))
        xpool = ctx.enter_context(tc.tile_pool(name="x", bufs=3))
        xvpool = ctx.enter_context(tc.tile_pool(name="xv", bufs=3))
        xrpool = ctx.enter_context(tc.tile_pool(name="xr", bufs=2))
        apool = ctx.enter_context(tc.tile_pool(name="act", bufs=1))
        espool = ctx.enter_context(tc.tile_pool(name="es", bufs=8))
        onpool = ctx.enter_context(tc.tile_pool(name="on", bufs=4))
        outp = ctx.enter_context(tc.tile_pool(name="out", bufs=3))
        rpool = ctx.enter_context(tc.tile_pool(name="r", bufs=4))
        oupool = ctx.enter_context(tc.tile_pool(name="ou", bufs=4))
        pps = ctx.enter_context(tc.tile_pool(name="pps", bufs=2, space="PSUM"))
        ppo = ctx.enter_context(tc.tile_pool(name="ppo", bufs=4, space="PSUM"))

        # ---- PE warmup: dummy matmuls to lift HAM to 8/8 ----
        wu = wpool.tile([128, 512], BF, tag="wu")
        nc.vector.memset(wu, 0.0)
        pwu = pps.tile([128, 1024], F, tag="ps")
        for i in range(22):
            nc.tensor.matmul(pwu[:, 0:512], lhsT=wu[:, 0:128], rhs=wu,
                             start=(i == 0), stop=(i == 21))

        # ---- constants / weights ----
        def loadw(name, shape):
            rt = wpool.tile(list(shape), BF, tag=name, name=name + "_t")
            nc.sync.dma_start(out=rt, in_=d[name])
            return rt

        def loadw_r(name, shape):
            raw = wpool.tile(list(shape), F, tag=name + "_raw", name=name + "_raw")
            nc.sync.dma_start(out=raw, in_=d[name])
            rt = wpool.tile(list(shape), R, tag=name, name=name + "_t")
            nc.vector.tensor_copy(out=rt, in_=raw)
            return rt

        wq = loadw_r("wq", (128, CI, 128))
        bqp = wpool.tile([128, 1], F, tag="bqp")
        nc.sync.dma_start(out=bqp, in_=d["bqp"])
        onesf = wpool.tile([1, 128], F, tag="onesf")
        nc.vector.memset(onesf, 1.0)
        ones = wpool.tile([1, 128], BF, tag="ones")
        nc.gpsimd.tensor_copy(out=ones, in_=onesf)
        ones_r = wpool.tile([1, 128], R, tag="ones_r")
        nc.vector.tensor_copy(out=ones_r, in_=onesf)
        onescol_f = wpool.tile([128, 1], F, tag="onescol_f")
        nc.vector.memset(onescol_f, 1.0)

        # persistent activation tiles
        q_sb = apool.tile([128, N], R, tag="q_sb")
        k_sb = apool.tile([128, N], R, tag="k_sb")
        v_t = [
            apool.tile([128, 256], R, tag=f"v{kt}", name=f"v{kt}")
            for kt in range(KT)
        ]

        def load_chunks(xname, g):
            if xname == "xvT":
                raw = xvpool.tile([128, CI, 512], BF, tag="xv", name="xv_raw")
                nc.sync.dma_start(out=raw, in_=d[xname][g].rearrange("ci p c -> p ci c"))
                return [raw[:, ci, :] for ci in range(CI)]
            raw = xpool.tile([128, CI, 512], F, tag="x", name="x_raw")
            nc.sync.dma_start(out=raw, in_=d[xname][g].rearrange("ci p c -> p ci c"))
            rnd = xrpool.tile([128, CI, 512], R, tag="xr", name="x_rnd")
            nc.vector.tensor_copy(out=rnd, in_=raw)
            return [rnd[:, ci, :] for ci in range(CI)]

        def proj_qk(xname, g, wt, bias, dst):
            chunks = load_chunks(xname, g)
            pq = ppo.tile([128, 512], F, tag="po", name="pq")
            for ci in range(CI):
                nc.tensor.matmul(pq, lhsT=wt[:, ci, :], rhs=chunks[ci],
                                 start=(ci == 0), stop=(ci == CI - 1))
            nc.vector.tensor_scalar_add(dst[:, g * 512:(g + 1) * 512], pq, bias)

        def proj_v(g):
            chunks = load_chunks("xvT", g)
            for j in range(4):
                kt = 4 * g + j
                nc.vector.tensor_copy(out=v_t[kt][:, 64:65], in_=onescol_f)
                nc.vector.tensor_copy(out=v_t[kt][:, 192:193], in_=onescol_f)
                pv = ppo.tile([128, 128], F, tag="po", name="pv")
                for ci in range(CI):
                    nc.tensor.matmul(pv, lhsT=chunks[ci][:, j * 128:(j + 1) * 128],
                                     rhs=wv[:, ci, :], start=(ci == 0), stop=(ci == CI - 1))
                nc.vector.tensor_add(v_t[kt][:, 0:64], pv[:, 0:64], bvb[:, 0:64])
                nc.vector.tensor_add(v_t[kt][:, 128:192], pv[:, 64:128], bvb[:, 64:128])

        def attn_block(qt, kts, po1, po2):
            qs = slice(qt * 512, (qt + 1) * 512)
            for kt in kts:
                ks = slice(kt * 128, (kt + 1) * 128)
                ps = pps.tile([128, 1024], F, tag="ps")
                nc.tensor.matmul(ps[:, 0:512], lhsT=k_sb[0:64, ks], rhs=q_sb[0:64, qs],
                                 start=True, stop=True)
                nc.tensor.matmul(ps[:, 512:1024], lhsT=k_sb[64:128, ks],
                                 rhs=q_sb[64:128, qs], start=True, stop=True)
                es = espool.tile([128, 1024], R, tag="es")
                nc.scalar.activation(out=es, in_=ps, func=EXPF,
                                     bias=mb[:, kt:kt + 1], scale=SCALE)
                nc.tensor.matmul(po1[0:65, :], lhsT=v_t[kt][:, 0:65], rhs=es[:, 0:512],
                                 start=(kt == 0), stop=(kt == KT - 1))
                nc.tensor.matmul(po2[0:65, :], lhsT=v_t[kt][:, 128:193], rhs=es[:, 512:1024],
                                 start=(kt == 0), stop=(kt == KT - 1))

        def finish_block(po1, po2):
            outs = []
            for po in (po1, po2):
                o_un = oupool.tile([64, 512], F, tag="ou")
                nc.vector.tensor_copy(o_un, po[0:64, :])
                se = rpool.tile([1, 512], F, tag="se")
                nc.vector.tensor_copy(se, po[64:65, :])
                outs.append((o_un, se))
            return outs

        def attn_tail(qt, evac):
            qs = slice(qt * 512, (qt + 1) * 512)
            on = onpool.tile([128, 512], R, tag="on")
            for h, (o_un, se) in enumerate(evac):
                rcf = rpool.tile([1, 512], F, tag="rcf")
                nc.vector.reciprocal_approx_fast(out=rcf, in_=se)
                rc = rpool.tile([1, 512], R, tag="rc")
                nc.vector.tensor_copy(out=rc, in_=rcf)
                pr = ppo.tile([65, 512], F, tag="po", name="pr")
                nc.tensor.matmul(pr, lhsT=ones_r[0:1, 0:65], rhs=rc, start=True, stop=True)
                nc.vector.tensor_mul(on[h * 64:(h + 1) * 64, :], o_un, pr[0:64, :])
            for ct in range(CI):
                cs = slice(ct * 128, (ct + 1) * 128)
                pz = ppo.tile([128, 512], F, tag="po", name="pz")
                nc.tensor.matmul(pz, lhsT=wo[:, cs], rhs=on, start=True, stop=True)
                ot = outp.tile([128, 512], F, tag="ot")
                if qt == QT - 1:
                    nc.scalar.add(ot, pz, bop[:, ct:ct + 1])
                else:
                    nc.vector.tensor_scalar_add(ot, pz, bop[:, ct:ct + 1])
                nc.sync.dma_start(out=outT[cs, qs], in_=ot)

        # ---- pipelined emission ----
        proj_qk("xqT", 0, wq, bqp, q_sb)
        wk = loadw_r("wk", (128, CI, 128))
        bkp = wpool.tile([128, 1], F, tag="bkp")
        nc.sync.dma_start(out=bkp, in_=d["bkp"])
        mb = wpool.tile([128, KT], F, tag="mb")
        nc.sync.dma_start(out=mb, in_=d["mb"])
        wv = loadw("wv", (128, CI, 128))
        bvp = loadw("bvp", (1, 128))
        # bv broadcast tile: [128, 128] every row = bv (via one K=1 matmul)
        pbv = ppo.tile([128, 128], F, tag="po", name="pbv")
        nc.tensor.matmul(pbv, lhsT=ones, rhs=bvp, start=True, stop=True)
        bvb = wpool.tile([128, 128], F, tag="bvb")
        nc.vector.tensor_copy(bvb, pbv)
        po1 = ppo.tile([128, 512], F, tag="po")
        po2 = ppo.tile([128, 512], F, tag="po")
        for g in range(QT):
            proj_qk("xkT", g, wk, bkp, k_sb)
            proj_v(g)
            if g >= 1:
                proj_qk("xqT", g, wq, bqp, q_sb)
            attn_block(0, range(4 * g, 4 * g + 4), po1, po2)
            if g == 0:
                wo = loadw_r("wo", (128, C))
                bop = wpool.tile([128, QT], F, tag="bop")
                nc.sync.dma_start(out=bop, in_=d["bop"])
        pending = (0, finish_block(po1, po2))
        for qt in range(1, QT):
            po1 = ppo.tile([128, 512], F, tag="po")
            po2 = ppo.tile([128, 512], F, tag="po")
            attn_block(qt, range(0, 4), po1, po2)
            attn_tail(*pending)
            attn_block(qt, range(4, KT), po1, po2)
            pending = (qt, finish_block(po1, po2))
        attn_tail(*pending)

    nc.compile()
    return nc


def get_nc():
    if "nc" not in _NC_CACHE:
        _NC_CACHE["nc"] = _build()
    return _NC_CACHE["nc"]


def shard_inputs(query, key, value, key_padding_mask, Wq, bq, Wk, bk, Wv, bv, Wo, bo):
    """Full inputs -> list of 8 per-core input dicts (host-side layout prep only)."""
    in_maps = []
    f32 = np.float32

    import ml_dtypes
    bf16h = ml_dtypes.bfloat16

    def gmajor(x, dt):
        # x (N, C) -> x.T (C, N) -> blocks (QT, CI, 128, 512):
        # [g, ci, p, c] = x.T[ci*128+p, g*512+c]
        xt = np.asarray(x).T.astype(dt)  # (512, 2048)
        return np.ascontiguousarray(
            xt.reshape(CI, 128, QT, 512).transpose(2, 0, 1, 3)
        )
    for c in range(8):
        b, hp = c // 4, c % 4
        rows = slice(hp * 128, (hp + 1) * 128)

        def wtile(W, dt):
            # (512, 128) slice of W.T -> (128, CI, 128): [p, ci, j] = W.T[ci*128+p, j]
            t = np.ascontiguousarray(np.asarray(W)[rows, :].T.astype(dt))
            return np.ascontiguousarray(t.reshape(CI, 128, 128).transpose(1, 0, 2))

        wo_p = Wo[:, rows].T.astype(f32)  # (128, C): rows = head-pair dims
        mbv = np.where(key_padding_mask[b], f32(NEG), f32(0.0)).astype(f32)
        in_maps.append({
            "xqT": gmajor(query[b], f32),
            "xkT": gmajor(key[b], f32),
            "xvT": gmajor(value[b], bf16h),
            "wq": wtile(Wq, f32), "wk": wtile(Wk, f32), "wv": wtile(Wv, bf16h),
            "wo": np.ascontiguousarray(wo_p.astype(f32)),
            "bqp": np.ascontiguousarray(bq[rows].astype(f32).reshape(128, 1)),
            "bkp": np.ascontiguousarray(bk[rows].astype(f32).reshape(128, 1)),
            "bvp": np.ascontiguousarray(bv[rows].astype(bf16h).reshape(1, 128)),
            "bop": np.ascontiguousarray(
                (bo.astype(f32) if hp == 0 else np.zeros(C, f32)).reshape(QT, 128).T
            ),
            "mb": np.ascontiguousarray(mbv.reshape(KT, 128).T),
        })
    return in_maps


def unshard_outputs(results):
    out = np.empty((B, N, C), np.float32)
    for b in range(B):
        acc = results[4 * b]["outT"].astype(np.float32).copy()
        for i in range(1, 4):
            acc += results[4 * b + i]["outT"]
        out[b] = acc.T
    return out


def kernel(**inputs):
    nc = get_nc()
    in_maps = shard_inputs(**{k: np.asarray(v) for k, v in inputs.items()})
    res = bass_utils.run_bass_kernel_spmd(nc, in_maps, core_ids=list(range(8)))
    return unshard_outputs(res.results)

